# revision 5
# baseline (speedup 1.0000x reference)
"""MelSpectrogram + PCEN Trainium2 kernel (8-core data parallel), v2.

Pipeline per core (8 batch elements):
  host: reflect-pad, hop-block transpose, time-domain butterfly
        pre-combination (b0/b2 for even f-planes, a- for odd), fp16
  PE:   hop-frame DFT via matmul (16 f-tiles x 4 rc chunks, W=512 cols)
        - even planes (p=0,2): rhs b0/b2 -> output is the final frame
          transform X directly (butterfly folded into time domain)
        - odd planes (p=1,3): rhs a- -> output Ys; X-step on DVE
  ACT:  PSUM->SBUF eviction (fp16 cast)
  DVE:  odd-plane X-step (X = Ys_t +/- Ys'_{t+1})
  DMA:  partition-shift copies for the cross-plane conv neighbors
  DVE/GPSIMD: frequency-domain hann 3-tap conv via scalar_tensor_tensor
  DVE/ACT: square -> power
  PE:   mel projection (fb folded with comp-duplication + scale)
  PE/ACT/DVE: PCEN per batch element, interleaved with chunk compute

f-slot layout: f = 4q + p, f in 0..1023 (f=1024 dropped; its mel weight
is 0 and its only use was the f=1023 conv neighbor - accepted error).
16 tiles of 128 q-slots: tiles 0-7 even planes [r0,r0,r2,r2,i0,i0,i2,i2],
tiles 8-15 odd planes [r1,r1,r3,r3,i1,i1,i3,i3]. (r=cos, i=-sin comps.)
"""

import math
from contextlib import ExitStack

import numpy as np

SR, N_FFT, HOP, N_MELS = 32000, 2048, 512, 128
F_MIN, F_MAX = 20.0, 16000.0
EPS, S, ALPHA, DELTA, R = 1e-6, 0.025, 0.98, 2.0, 0.5
NBINS = N_FFT // 2 + 1
T = 313          # frames per batch element
SBLK = 316       # hop blocks per batch element
PAD = N_FFT // 2
B_TOTAL, L_WAVE = 64, 160000
N_CORES = 8

SC = 16.0    # E scale (E = E_true/SC)
SCM = 16.0   # mel scale (mel' = mel_true/SCM)
SCL = 8.0    # L scale (LT = L*SCL)
SCE = 256.0  # e2 scale (keeps (eps+m)^-alpha in fp16 normal range)

W = 512            # chunk width (max matmul moving dim)
STRIDE = W - 3
NC = 8 * SBLK      # 2528 blocks per core
NC_PAD = 2560      # padded dram cols
NTILE = 16

# tile start index per (comp, plane)
TSTART = {("r", 0): 0, ("r", 2): 2, ("i", 0): 4, ("i", 2): 6,
          ("r", 1): 8, ("r", 3): 10, ("i", 1): 12, ("i", 3): 14}
TILES16 = [("r", 0), ("r", 0), ("r", 2), ("r", 2),
           ("i", 0), ("i", 0), ("i", 2), ("i", 2),
           ("r", 1), ("r", 1), ("r", 3), ("r", 3),
           ("i", 1), ("i", 1), ("i", 3), ("i", 3)]


def _chunks():
    out = []
    co = 0
    while co + 3 < NC:
        w = min(W, NC - co)
        out.append((co, w))
        if co + w >= NC:
            break
        co += STRIDE
    return out


CHUNKS = _chunks()


def _slot(f, comp):
    p, q = f % 4, f // 4
    return TSTART[(comp, p)] * 128 + q


def _mel_fbank():
    def hz2mel(f):
        return 2595.0 * np.log10(1.0 + np.asarray(f, np.float64) / 700.0)

    def mel2hz(m):
        return 700.0 * (10.0 ** (np.asarray(m, np.float64) / 2595.0) - 1.0)

    all_freqs = np.linspace(0.0, SR / 2.0, NBINS)
    m_pts = np.linspace(hz2mel(F_MIN), hz2mel(F_MAX), N_MELS + 2)
    f_pts = mel2hz(m_pts)
    f_diff = np.diff(f_pts)
    slopes = f_pts[None, :] - all_freqs[:, None]
    down = -slopes[:, :-2] / f_diff[:-1]
    up = slopes[:, 2:] / f_diff[1:]
    return np.maximum(0.0, np.minimum(down, up))  # [1025, 128]


def _build_consts():
    r = np.arange(512)
    E = np.zeros((512, NTILE * 128), np.float64)
    for f in range(1024):
        th = 2.0 * np.pi * f * r / N_FFT
        E[:, _slot(f, "r")] = np.cos(th) / SC
        E[:, _slot(f, "i")] = -np.sin(th) / SC
    fb = _mel_fbank()
    fb2 = np.zeros((NTILE * 128, N_MELS), np.float64)
    for f in range(1024):
        wgt = fb[f] * (SC * SC / 4.0) / SCM
        fb2[_slot(f, "r")] = wgt
        fb2[_slot(f, "i")] = wgt
    LT = np.zeros((384, T), np.float64)
    t = np.arange(T)
    for tau in range(T):
        m = t >= tau
        LT[tau, m] = S * (1.0 - S) ** (t[m] - tau) * SCL
    return E, fb2, LT


def _split_multiwaits(nc, limit=1):
    """This walrus build accepts at most `limit` sync-waits per instruction;
    move excess waits onto preceding same-engine NoOps."""
    import bass_rust
    import concourse.mybir as mybir

    for fn in nc.m.functions:
        for b in fn.blocks:
            insts = b.instructions
            new = []
            changed = False
            for i in insts:
                si = i.sync_info
                if si is not None and len(si.on_wait) > limit:
                    waits = list(si.on_wait)
                    for k in range(0, len(waits) - limit, limit):
                        chunk = waits[k : k + limit]
                        nop = mybir.InstNoOp(
                            name=f"{i.name}-wsplit{k}", ins=[], outs=[]
                        )
                        nop.engine = i.engine
                        nop.sync_info = bass_rust.SyncInfo(
                            on_wait=chunk, on_update=[]
                        )
                        new.append(nop)
                        changed = True
                    si.on_wait = waits[len(waits) - limit :]
                new.append(i)
            if changed:
                b.instructions = new


def build_nc(BL=8, split=True):
    import concourse.bass as bass
    import concourse.mybir as mybir
    from concourse import tile

    f16 = mybir.dt.float16
    f32 = mybir.dt.float32
    ALU = mybir.AluOpType
    ACTF = mybir.ActivationFunctionType

    nc = bass.Bass("TRN2", target_bir_lowering=False, debug=False)
    b0_d = nc.dram_tensor("xb0", [4, 128, NC_PAD], f16, kind="ExternalInput")
    b2_d = nc.dram_tensor("xb2", [4, 128, NC_PAD], f16, kind="ExternalInput")
    an_d = nc.dram_tensor("xan", [4, 128, NC_PAD], f16, kind="ExternalInput")
    e_d = nc.dram_tensor("e", [4, 128, NTILE * 128], f16, kind="ExternalInput")
    fb_d = nc.dram_tensor("fb", [128, NTILE * 128], f16, kind="ExternalInput")
    lt_d = nc.dram_tensor("lt", [3, 128, T], f16, kind="ExternalInput")
    id_d = nc.dram_tensor("idn", [128, 128], f16, kind="ExternalInput")
    y_d = nc.dram_tensor("y", [BL, 128, T], f32, kind="ExternalOutput")

    nch = len(CHUNKS)
    # after chunk ci (frames <= co+V-1), these batches are complete
    TAIL_AFTER = {ci: [] for ci in range(nch)}
    fdone = 0
    bnext = 0
    for ci, (co, w) in enumerate(CHUNKS):
        V = min(w, NC - co) - 3
        fdone = co + V - 1
        while bnext < BL and bnext * SBLK + (T - 1) <= fdone:
            TAIL_AFTER[ci].append(bnext)
            bnext += 1
    assert bnext == BL, f"tail scheduling incomplete {bnext}"

    with tile.TileContext(nc) as tc, ExitStack() as top:
        cpool = top.enter_context(tc.tile_pool(name="consts", bufs=1))
        eb = cpool.tile([128, 4 * NTILE * 128], f16)
        fbb = cpool.tile([128, NTILE * 128], f16)
        ltb = cpool.tile([128, 3 * T], f16)
        idb = cpool.tile([128, 128], f16)
        melb = cpool.tile([128, NC], f16)
        bias_t = cpool.tile([128, 3], f32)
        zrow = cpool.tile([128, W], f16)
        nc.vector.memset(zrow[:, :], 0.0)

        ebv = eb[:, :].rearrange("p (rc c) -> p rc c", rc=4)
        for rc in range(4):
            nc.sync.dma_start(ebv[:, rc, :], e_d.ap()[rc])
        nc.sync.dma_start(fbb[:, :], fb_d.ap()[:, :])
        ltv = ltb[:, :].rearrange("p (k t) -> p k t", k=3)
        for k in range(3):
            nc.sync.dma_start(ltv[:, k, :], lt_d.ap()[k])
        nc.sync.dma_start(idb[:, :], id_d.ap()[:, :])
        nc.vector.memset(bias_t[:, 0:1], EPS)
        nc.vector.memset(bias_t[:, 1:2], math.log(SCE))
        nc.vector.memset(bias_t[:, 2:3], DELTA)

        with ExitStack() as cph:
            yps = cph.enter_context(tc.tile_pool(name="yps", bufs=3, space="PSUM"))
            mps = cph.enter_context(tc.tile_pool(name="mps", bufs=2, space="PSUM"))
            tps = cph.enter_context(tc.tile_pool(name="tps", bufs=1, space="PSUM"))
            msps = cph.enter_context(tc.tile_pool(name="msp", bufs=1, space="PSUM"))
            p_rhs = cph.enter_context(tc.tile_pool(name="rhs", bufs=2))
            p_ysb = cph.enter_context(tc.tile_pool(name="ysb", bufs=2))
            p_x = cph.enter_context(tc.tile_pool(name="xbuf", bufs=2))
            p_sh = cph.enter_context(tc.tile_pool(name="sh", bufs=2))
            p_xw = cph.enter_context(tc.tile_pool(name="xw", bufs=1))
            p_pw = cph.enter_context(tc.tile_pool(name="pw", bufs=2))
            p_tail = cph.enter_context(tc.tile_pool(name="tail", bufs=2))

            def emit_prep(ci):
                """DMA this chunk's rhs slices (b0/b2/a-) into SBUF."""
                co, w = CHUNKS[ci]
                tiles = {}
                for name, dram in (("b0", b0_d), ("b2", b2_d), ("an", an_d)):
                    tl = p_rhs.tile([128, 4 * W], f16, tag=name)
                    tv = tl[:, :].rearrange("p (rc c) -> p rc c", rc=4)
                    for rc in range(4):
                        nc.sync.dma_start(
                            tv[:, rc, 0:w], dram.ap()[rc][:, co : co + w]
                        )
                    tiles[name] = tl
                return tiles

            RHS_P = {0: "b0", 2: "b2", 1: "an", 3: "an"}

            def emit_dft(ci, rhs):
                co, w = CHUNKS[ci]
                ysb = p_ysb.tile([128, 8 * W], f16, tag="ysb")
                xbf = p_x.tile([128, NTILE * W], f16, tag="x")
                for j in range(NTILE):
                    comp, p = TILES16[j]
                    rt = rhs[RHS_P[p]]
                    rtv = rt[:, :].rearrange("p (rc c) -> p rc c", rc=4)
                    yp = yps.tile([128, W], f32, tag="yp")
                    for rc in range(4):
                        nc.tensor.matmul(
                            yp[:, 0:w],
                            ebv[:, rc, j * 128 : (j + 1) * 128],
                            rtv[:, rc, 0:w],
                            start=(rc == 0),
                            stop=(rc == 3),
                        )
                    if j < 8:
                        dst = xbf[:, j * W : j * W + w]
                    else:
                        dst = ysb[:, (j - 8) * W : (j - 8) * W + w]
                    if j in (7, 15):
                        nc.vector.tensor_copy(dst, yp[:, 0:w])
                    else:
                        nc.scalar.copy(dst, yp[:, 0:w])
                return ysb, xbf

            def emit_post(ci, ysb, xbf):
                co, w = CHUNKS[ci]
                V = min(w, NC - co) - 3
                L = 2 * W - 1
                add = nc.vector.tensor_add
                sub = nc.vector.tensor_sub
                # odd-plane X-step; ysb groups 0-1:r1 2-3:r3 4-5:i1 6-7:i3
                add(xbf[:, 8 * W : 8 * W + L], ysb[:, 0:L],
                    ysb[:, 4 * W + 1 : 4 * W + 1 + L])                 # Xr1
                sub(xbf[:, 12 * W : 12 * W + L], ysb[:, 4 * W : 4 * W + L],
                    ysb[:, 1 : 1 + L])                                 # Xi1
                sub(xbf[:, 10 * W : 10 * W + L], ysb[:, 2 * W : 2 * W + L],
                    ysb[:, 6 * W + 1 : 6 * W + 1 + L])                 # Xr3
                add(xbf[:, 14 * W : 14 * W + L], ysb[:, 6 * W : 6 * W + L],
                    ysb[:, 2 * W + 1 : 2 * W + 1 + L])                 # Xi3

                # cross-plane conv neighbors via DMA partition shifts
                shs = {}
                for comp in ("r", "i"):
                    p0lo = TSTART[(comp, 0)]
                    p3lo = TSTART[(comp, 3)]
                    S3 = p_sh.tile([128, 2 * W], f16, tag=f"s3{comp}")
                    nc.sync.dma_start(
                        S3[1:128, :], xbf[0:127, p3lo * W : (p3lo + 2) * W]
                    )
                    nc.sync.dma_start(
                        S3[0:1, W : 2 * W],
                        xbf[127:128, p3lo * W : (p3lo + 1) * W],
                    )
                    nc.gpsimd.memset(S3[0:1, 0:W], 0.0)
                    S0 = p_sh.tile([128, 2 * W], f16, tag=f"s0{comp}")
                    nc.sync.dma_start(
                        S0[0:127, :], xbf[1:128, p0lo * W : (p0lo + 2) * W]
                    )
                    nc.sync.dma_start(
                        S0[127:128, 0:W],
                        xbf[0:1, (p0lo + 1) * W : (p0lo + 2) * W],
                    )
                    nc.sync.dma_start(S0[127:128, W : 2 * W], zrow[0:1, 0:W])
                    shs[comp] = (S3, S0)

                # hann conv: xw_p = X_p - 0.5*(X_{p-1} + X_{p+1})
                xw = p_xw.tile([128, NTILE * W], f16, tag="xw")
                M, A = ALU.mult, ALU.add

                def RX(t0):
                    return xbf[:, t0 * W : (t0 + 2) * W]

                def RW(t0):
                    return xw[:, t0 * W : (t0 + 2) * W]

                for comp, gps_planes in (("r", (1, 2)), ("i", (1,))):
                    S3, S0 = shs[comp]
                    p0 = TSTART[(comp, 0)]
                    p2 = TSTART[(comp, 2)]
                    p1 = TSTART[(comp, 1)]
                    p3 = TSTART[(comp, 3)]
                    nbr = {0: (S3[:, :], RX(p1)), 1: (RX(p0), RX(p2)),
                           2: (RX(p1), RX(p3)), 3: (RX(p2), S0[:, :])}
                    ctr = {0: RX(p0), 1: RX(p1), 2: RX(p2), 3: RX(p3)}
                    out = {0: RW(p0), 1: RW(p1), 2: RW(p2), 3: RW(p3)}
                    for p in range(4):
                        # Pool engine rejects TensorScalarPtr in codegen:
                        # adds may go to gpsimd, stt stays on vector
                        eng = nc.gpsimd if p in gps_planes else nc.vector
                        lo, hi = nbr[p]
                        eng.tensor_add(out[p], lo, hi)
                        nc.vector.scalar_tensor_tensor(
                            out[p], out[p], -0.5, ctr[p], M, A
                        )

                # power
                pw = p_pw.tile([128, NTILE * W], f16, tag="pw")
                hsplit = 8 * W
                nc.vector.tensor_mul(
                    pw[:, 0:hsplit], xw[:, 0:hsplit], xw[:, 0:hsplit]
                )
                nc.scalar.activation(
                    pw[:, hsplit:], xw[:, hsplit:], ACTF.Square
                )

                # mel projection
                mp = mps.tile([128, W], f32, tag="mp")
                for j in range(NTILE):
                    nc.tensor.matmul(
                        mp[:, 0:V],
                        fbb[:, j * 128 : (j + 1) * 128],
                        pw[:, j * W : j * W + V],
                        start=(j == 0),
                        stop=(j == NTILE - 1),
                    )
                nc.scalar.copy(melb[:, co : co + V], mp[:, 0:V])

            tlens = (128, 128, 57)

            def emit_tail(b):
                melT = p_tail.tile([128, 3 * 128], f16, tag="melT")
                for k in range(3):
                    tl = tlens[k]
                    tp = tps.tile([128, 128], f16, tag="tp")
                    nc.tensor.transpose(
                        tp[0:tl, :],
                        melb[:, b * SBLK + k * 128 : b * SBLK + k * 128 + tl],
                        idb[:, :],
                    )
                    nc.vector.tensor_copy(
                        melT[0:tl, k * 128 : (k + 1) * 128], tp[0:tl, :]
                    )
                msp = msps.tile([128, T], f32, tag="ms")
                for k in range(3):
                    tl = tlens[k]
                    nc.tensor.matmul(
                        msp[:, :],
                        melT[0:tl, k * 128 : (k + 1) * 128],
                        ltv[0:tl, k, :],
                        start=(k == 0),
                        stop=(k == 2),
                    )
                e1 = p_tail.tile([128, T], f32, tag="e1")
                e2 = p_tail.tile([128, T], f32, tag="e2")
                e3 = p_tail.tile([128, T], f16, tag="e3")
                e4 = p_tail.tile([128, T], f32, tag="e4")
                e5 = p_tail.tile([128, T], f32, tag="e5")
                ob = p_tail.tile([128, T], f32, tag="ob")
                # e1 = ln(ms*SCM/SCL + EPS)
                nc.scalar.activation(
                    e1[:, :], msp[:, :], ACTF.Ln,
                    bias=bias_t[:, 0:1], scale=SCM / SCL,
                )
                # e2 = exp(-alpha*e1 + ln(SCE)) = SCE*(eps+m)^-alpha
                nc.scalar.activation(
                    e2[:, :], e1[:, :], ACTF.Exp,
                    bias=bias_t[:, 1:2], scale=-ALPHA,
                )
                nc.vector.tensor_mul(
                    e3[:, :], e2[:, :], melb[:, b * SBLK : b * SBLK + T]
                )
                # e4 = ln(e3*SCM/SCE + DELTA)
                nc.scalar.activation(
                    e4[:, :], e3[:, :], ACTF.Ln,
                    bias=bias_t[:, 2:3], scale=SCM / SCE,
                )
                nc.scalar.activation(e5[:, :], e4[:, :], ACTF.Exp, scale=R)
                nc.vector.tensor_scalar_add(ob[:, :], e5[:, :], -(DELTA ** R))
                nc.sync.dma_start(y_d.ap()[b], ob[:, :])

            saved = {}
            saved[0] = emit_dft(0, emit_prep(0))
            saved[1] = emit_dft(1, emit_prep(1))
            for ci in range(nch):
                emit_post(ci, *saved.pop(ci))
                for b in TAIL_AFTER[ci]:
                    emit_tail(b)
                if ci + 2 < nch:
                    saved[ci + 2] = emit_dft(ci + 2, emit_prep(ci + 2))

    if split:
        _split_multiwaits(nc)
    return nc


# ---------------------------------------------------------------- host side

_CACHE = {}


def _get_consts():
    if "consts" not in _CACHE:
        E, fb2, LT = _build_consts()
        e_h = np.ascontiguousarray(
            E.astype(np.float16).reshape(4, 128, NTILE * 128)
        )
        # fb tile layout: fb_h[p, j*128+m] = fb2[j*128+p, m]
        fb_h = np.ascontiguousarray(
            fb2.astype(np.float16)
            .reshape(NTILE, 128, 128)
            .transpose(1, 0, 2)
            .reshape(128, NTILE * 128)
        )
        lt_h = np.ascontiguousarray(
            LT.astype(np.float16).reshape(3, 128, T)
        )
        id_h = np.eye(128, dtype=np.float16)
        _CACHE["consts"] = (e_h, fb_h, lt_h, id_h)
    return _CACHE["consts"]


def _prep_core_input(wf_core):
    """wf_core: [BL, 160000] f32 -> (b0, b2, an) each [4, 128, NC_PAD] f16.

    Time-domain butterfly pre-combination over hop blocks t:
      b0_t = x_t + x_{t+1} + x_{t+2} + x_{t+3}   (even f-planes, p=0)
      b2_t = x_t - x_{t+1} + x_{t+2} - x_{t+3}   (even f-planes, p=2)
      an_t = x_t - x_{t+2}                        (odd f-planes)
    """
    BL = wf_core.shape[0]
    x = np.pad(wf_core.astype(np.float32), ((0, 0), (PAD, PAD)), mode="reflect")
    blocks = x[:, : SBLK * HOP].reshape(BL, SBLK, HOP)
    xT = blocks.transpose(2, 0, 1).reshape(HOP, BL * SBLK)
    g = np.zeros((HOP, NC + 3), np.float32)
    g[:, :NC] = xT
    outs = []
    for arr in (
        g[:, 0:NC] + g[:, 1 : NC + 1] + g[:, 2 : NC + 2] + g[:, 3 : NC + 3],
        g[:, 0:NC] - g[:, 1 : NC + 1] + g[:, 2 : NC + 2] - g[:, 3 : NC + 3],
        g[:, 0:NC] - g[:, 2 : NC + 2],
    ):
        h = np.zeros((HOP, NC_PAD), np.float16)
        h[:, :NC] = arr.astype(np.float16)
        outs.append(np.ascontiguousarray(h.reshape(4, 128, NC_PAD)))
    return outs


def make_in_maps(waveform):
    e_h, fb_h, lt_h, id_h = _get_consts()
    BL = B_TOTAL // N_CORES
    in_maps = []
    for c in range(N_CORES):
        b0, b2, an = _prep_core_input(waveform[c * BL : (c + 1) * BL])
        in_maps.append(
            {"xb0": b0, "xb2": b2, "xan": an, "e": e_h, "fb": fb_h,
             "lt": lt_h, "idn": id_h}
        )
    return in_maps


def _get_nc():
    if "nc" not in _CACHE:
        _CACHE["nc"] = build_nc(BL=8)
    return _CACHE["nc"]


def kernel(waveform: np.ndarray) -> np.ndarray:
    from concourse.bass_utils import run_bass_kernel_spmd

    waveform = np.asarray(waveform, np.float32)
    assert waveform.shape == (B_TOTAL, L_WAVE)
    in_maps = make_in_maps(waveform)
    nc = _get_nc()
    res = run_bass_kernel_spmd(nc, in_maps, core_ids=list(range(N_CORES)))
    BL = B_TOTAL // N_CORES
    out = np.empty((B_TOTAL, 1, N_MELS, T), np.float32)
    for c in range(N_CORES):
        y = np.asarray(res.results[c]["y"])  # [BL, 128, T]
        out[c * BL : (c + 1) * BL, 0] = y
    return out


# revision 12
# speedup vs baseline: 1.2886x; 1.2886x over previous
"""MelSpectrogram + PCEN Trainium2 kernel (8-core data parallel), v2.

Pipeline per core (8 batch elements):
  host: reflect-pad, hop-block transpose, time-domain butterfly
        pre-combination (b0/b2 for even f-planes, a- for odd), fp16
  PE:   hop-frame DFT via matmul (16 f-tiles x 4 rc chunks, W=512 cols)
        - even planes (p=0,2): rhs b0/b2 -> output is the final frame
          transform X directly (butterfly folded into time domain)
        - odd planes (p=1,3): rhs a- -> output Ys; X-step on DVE
  ACT:  PSUM->SBUF eviction (fp16 cast)
  DVE:  odd-plane X-step (X = Ys_t +/- Ys'_{t+1})
  DMA:  partition-shift copies for the cross-plane conv neighbors
  DVE/GPSIMD: frequency-domain hann 3-tap conv via scalar_tensor_tensor
  DVE/ACT: square -> power
  PE:   mel projection (fb folded with comp-duplication + scale)
  PE/ACT/DVE: PCEN per batch element, interleaved with chunk compute

f-slot layout: f = 4q + p, f in 0..1023 (f=1024 dropped; its mel weight
is 0 and its only use was the f=1023 conv neighbor - accepted error).
16 tiles of 128 q-slots: tiles 0-7 even planes [r0,r0,r2,r2,i0,i0,i2,i2],
tiles 8-15 odd planes [r1,r1,r3,r3,i1,i1,i3,i3]. (r=cos, i=-sin comps.)
"""

import math
from contextlib import ExitStack

import numpy as np

SR, N_FFT, HOP, N_MELS = 32000, 2048, 512, 128
F_MIN, F_MAX = 20.0, 16000.0
EPS, S, ALPHA, DELTA, R = 1e-6, 0.025, 0.98, 2.0, 0.5
NBINS = N_FFT // 2 + 1
T = 313          # frames per batch element
SBLK = 316       # hop blocks per batch element
PAD = N_FFT // 2
B_TOTAL, L_WAVE = 64, 160000
N_CORES = 8

SC = 16.0    # E scale (E = E_true/SC)
SCM = 16.0   # mel scale (mel' = mel_true/SCM)
SCL = 8.0    # L scale (LT = L*SCL)
SCE = 256.0  # e2 scale (keeps (eps+m)^-alpha in fp16 normal range)

W = 512            # chunk width (max matmul moving dim)
STRIDE = W - 3
NC = 8 * SBLK      # 2528 blocks per core
SEG = 520          # x cols staged per chunk per rc (w + lookahead)
NTILE = 16

# tile start index per (comp, plane)
TSTART = {("r", 0): 0, ("r", 2): 2, ("i", 0): 4, ("i", 2): 6,
          ("r", 1): 8, ("r", 3): 10, ("i", 1): 12, ("i", 3): 14}
TILES16 = [("r", 0), ("r", 0), ("r", 2), ("r", 2),
           ("i", 0), ("i", 0), ("i", 2), ("i", 2),
           ("r", 1), ("r", 1), ("r", 3), ("r", 3),
           ("i", 1), ("i", 1), ("i", 3), ("i", 3)]


def _chunks():
    out = []
    co = 0
    while co + 3 < NC:
        w = min(W, NC - co)
        out.append((co, w))
        if co + w >= NC:
            break
        co += STRIDE
    return out


CHUNKS = _chunks()


def _slot(f, comp):
    p, q = f % 4, f // 4
    return TSTART[(comp, p)] * 128 + q


def _mel_fbank():
    def hz2mel(f):
        return 2595.0 * np.log10(1.0 + np.asarray(f, np.float64) / 700.0)

    def mel2hz(m):
        return 700.0 * (10.0 ** (np.asarray(m, np.float64) / 2595.0) - 1.0)

    all_freqs = np.linspace(0.0, SR / 2.0, NBINS)
    m_pts = np.linspace(hz2mel(F_MIN), hz2mel(F_MAX), N_MELS + 2)
    f_pts = mel2hz(m_pts)
    f_diff = np.diff(f_pts)
    slopes = f_pts[None, :] - all_freqs[:, None]
    down = -slopes[:, :-2] / f_diff[:-1]
    up = slopes[:, 2:] / f_diff[1:]
    return np.maximum(0.0, np.minimum(down, up))  # [1025, 128]


def _build_consts():
    r = np.arange(512)
    E = np.zeros((512, NTILE * 128), np.float64)
    for f in range(1024):
        th = 2.0 * np.pi * f * r / N_FFT
        E[:, _slot(f, "r")] = np.cos(th) / SC
        E[:, _slot(f, "i")] = -np.sin(th) / SC
    fb = _mel_fbank()
    fb2 = np.zeros((NTILE * 128, N_MELS), np.float64)
    for f in range(1024):
        wgt = fb[f] * (SC * SC / 4.0) / SCM
        fb2[_slot(f, "r")] = wgt
        fb2[_slot(f, "i")] = wgt
    LT = np.zeros((384, T), np.float64)
    t = np.arange(T)
    for tau in range(T):
        m = t >= tau
        LT[tau, m] = S * (1.0 - S) ** (t[m] - tau) * SCL
    return E, fb2, LT


def _split_multiwaits(nc, limit=1):
    """This walrus build accepts at most `limit` sync-waits per instruction;
    move excess waits onto preceding same-engine NoOps."""
    import bass_rust
    import concourse.mybir as mybir

    for fn in nc.m.functions:
        for b in fn.blocks:
            insts = b.instructions
            new = []
            changed = False
            for i in insts:
                si = i.sync_info
                if si is not None and len(si.on_wait) > limit:
                    waits = list(si.on_wait)
                    for k in range(0, len(waits) - limit, limit):
                        chunk = waits[k : k + limit]
                        nop = mybir.InstNoOp(
                            name=f"{i.name}-wsplit{k}", ins=[], outs=[]
                        )
                        nop.engine = i.engine
                        nop.sync_info = bass_rust.SyncInfo(
                            on_wait=chunk, on_update=[]
                        )
                        new.append(nop)
                        changed = True
                    si.on_wait = waits[len(waits) - limit :]
                new.append(i)
            if changed:
                b.instructions = new


def build_nc(BL=8, split=True):
    import concourse.bass as bass
    import concourse.mybir as mybir
    from concourse import tile

    f16 = mybir.dt.float16
    f32 = mybir.dt.float32
    ALU = mybir.AluOpType
    ACTF = mybir.ActivationFunctionType

    nc = bass.Bass("TRN2", target_bir_lowering=False, debug=False)
    nch_ = len(CHUNKS)
    xt_d = nc.dram_tensor(
        "xt2", [128, nch_ * 4 * SEG], f16, kind="ExternalInput"
    )
    e_d = nc.dram_tensor("e", [4, 128, NTILE * 128], f16, kind="ExternalInput")
    sh_d = nc.dram_tensor("sh", [128, 4 * 128], f16, kind="ExternalInput")
    fb_d = nc.dram_tensor("fb", [128, NTILE * 128], f16, kind="ExternalInput")
    lt_d = nc.dram_tensor("lt", [3, 128, T], f16, kind="ExternalInput")
    id_d = nc.dram_tensor("idn", [128, 128], f16, kind="ExternalInput")
    y_d = nc.dram_tensor("y", [BL, 128, T], f32, kind="ExternalOutput")

    nch = len(CHUNKS)
    # after chunk ci (frames <= co+V-1), these batches are complete
    TAIL_AFTER = {ci: [] for ci in range(nch)}
    fdone = 0
    bnext = 0
    for ci, (co, w) in enumerate(CHUNKS):
        V = min(w, NC - co) - 3
        fdone = co + V - 1
        while bnext < BL and bnext * SBLK + (T - 1) <= fdone:
            TAIL_AFTER[ci].append(bnext)
            bnext += 1
    assert bnext == BL, f"tail scheduling incomplete {bnext}"

    with tile.TileContext(nc) as tc, ExitStack() as top:
        cpool = top.enter_context(tc.tile_pool(name="consts", bufs=1))
        eb = cpool.tile([128, 4 * NTILE * 128], f16)
        fbb = cpool.tile([128, NTILE * 128], f16)
        ltb = cpool.tile([128, 3 * T], f16)
        idb = cpool.tile([128, 128], f16)
        melb = cpool.tile([128, NC], f16)
        bias_t = cpool.tile([128, 3], f32)
        shb = cpool.tile([128, 4 * 128], f16)

        ebv = eb[:, :].rearrange("p (rc c) -> p rc c", rc=4)
        for rc in range(4):
            nc.sync.dma_start(ebv[:, rc, :], e_d.ap()[rc])
        nc.sync.dma_start(fbb[:, :], fb_d.ap()[:, :])
        ltv = ltb[:, :].rearrange("p (k t) -> p k t", k=3)
        for k in range(3):
            nc.sync.dma_start(ltv[:, k, :], lt_d.ap()[k])
        nc.sync.dma_start(idb[:, :], id_d.ap()[:, :])
        nc.sync.dma_start(shb[:, :], sh_d.ap()[:, :])
        shv = shb[:, :].rearrange("p (k c) -> p k c", k=4)
        SDN, CFIRST, SUP, CLAST = 0, 1, 2, 3
        nc.vector.memset(bias_t[:, 0:1], EPS)
        nc.vector.memset(bias_t[:, 1:2], math.log(SCE))
        nc.vector.memset(bias_t[:, 2:3], DELTA)

        with ExitStack() as cph:
            yps = cph.enter_context(tc.tile_pool(name="yps", bufs=2, space="PSUM"))
            mps = cph.enter_context(tc.tile_pool(name="mps", bufs=1, space="PSUM"))
            xsps = cph.enter_context(
                tc.tile_pool(name="xsps", bufs=3, space="PSUM")
            )
            tps = cph.enter_context(tc.tile_pool(name="tps", bufs=1, space="PSUM"))
            msps = cph.enter_context(tc.tile_pool(name="msp", bufs=1, space="PSUM"))
            p_xc = cph.enter_context(tc.tile_pool(name="xc", bufs=2))
            p_rhs = cph.enter_context(tc.tile_pool(name="rhs", bufs=2))
            p_ysb = cph.enter_context(tc.tile_pool(name="ysb", bufs=2))
            p_x = cph.enter_context(tc.tile_pool(name="xbuf", bufs=2))
            p_xw = cph.enter_context(tc.tile_pool(name="xw", bufs=1))
            p_pw = cph.enter_context(tc.tile_pool(name="pw", bufs=2))
            p_tail = cph.enter_context(tc.tile_pool(name="tail", bufs=2))

            def emit_prep(ci):
                """Load this chunk's x slice, build b0/b2/a- on DVE/GPS.

                b0_t = a+_t + a+_{t+1}, b2_t = a+_t - a+_{t+1} (even planes),
                a-_t = x_t - x_{t+2} (odd planes), a+_t = x_t + x_{t+2}.
                """
                xc = p_xc.tile([128, 4 * SEG], f16, tag="xc")
                base = ci * 4 * SEG
                half = 2 * SEG
                nc.sync.dma_start(
                    xc[:, 0:half], xt_d.ap()[:, base : base + half]
                )
                nc.sync.dma_start(
                    xc[:, half : 4 * SEG],
                    xt_d.ap()[:, base + half : base + 4 * SEG],
                )
                ap_t = p_rhs.tile([128, 4 * SEG], f16, tag="ap")
                an_t = p_rhs.tile([128, 4 * SEG], f16, tag="an")
                b0_t = p_rhs.tile([128, 4 * SEG], f16, tag="b0")
                b2_t = p_rhs.tile([128, 4 * SEG], f16, tag="b2")
                n = 4 * SEG - 2
                nc.vector.tensor_add(ap_t[:, 0:n], xc[:, 0:n], xc[:, 2 : n + 2])
                nc.vector.tensor_sub(an_t[:, 0:n], xc[:, 0:n], xc[:, 2 : n + 2])
                n1 = n - 1
                nc.gpsimd.tensor_add(
                    b0_t[:, 0:n1], ap_t[:, 0:n1], ap_t[:, 1 : n1 + 1]
                )
                nc.gpsimd.tensor_sub(
                    b2_t[:, 0:n1], ap_t[:, 0:n1], ap_t[:, 1 : n1 + 1]
                )
                return {"b0": b0_t, "b2": b2_t, "an": an_t}

            RHS_P = {0: "b0", 2: "b2", 1: "an", 3: "an"}

            def emit_dft(ci, rhs):
                co, w = CHUNKS[ci]
                ysb = p_ysb.tile([128, 8 * W], f16, tag="ysb")
                xbf = p_x.tile([128, NTILE * W], f16, tag="x")
                for j in range(NTILE):
                    comp, p = TILES16[j]
                    rt = rhs[RHS_P[p]]
                    rtv = rt[:, :].rearrange("p (rc c) -> p rc c", rc=4)
                    yp = yps.tile([128, W], f32, tag="yp")
                    for rc in range(4):
                        nc.tensor.matmul(
                            yp[:, 0:w],
                            ebv[:, rc, j * 128 : (j + 1) * 128],
                            rtv[:, rc, 0:w],
                            start=(rc == 0),
                            stop=(rc == 3),
                        )
                    if j < 8:
                        dst = xbf[:, j * W : j * W + w]
                    else:
                        dst = ysb[:, (j - 8) * W : (j - 8) * W + w]
                    if j in (7, 15):
                        nc.vector.tensor_copy(dst, yp[:, 0:w])
                    else:
                        nc.scalar.copy(dst, yp[:, 0:w])
                return ysb, xbf

            # rhs tiles have SEG-strided rc segments; matmul reads cols 0..w-1
            # of each segment, which stay clear of the segment-tail garbage.

            def emit_post(ci, ysb, xbf):
                co, w = CHUNKS[ci]
                V = min(w, NC - co) - 3
                L = 2 * W - 1
                add = nc.vector.tensor_add
                sub = nc.vector.tensor_sub
                # odd-plane X-step; ysb groups 0-1:r1 2-3:r3 4-5:i1 6-7:i3
                add(xbf[:, 8 * W : 8 * W + L], ysb[:, 0:L],
                    ysb[:, 4 * W + 1 : 4 * W + 1 + L])                 # Xr1
                sub(xbf[:, 12 * W : 12 * W + L], ysb[:, 4 * W : 4 * W + L],
                    ysb[:, 1 : 1 + L])                                 # Xi1
                sub(xbf[:, 10 * W : 10 * W + L], ysb[:, 2 * W : 2 * W + L],
                    ysb[:, 6 * W + 1 : 6 * W + 1 + L])                 # Xr3
                add(xbf[:, 14 * W : 14 * W + L], ysb[:, 6 * W : 6 * W + L],
                    ysb[:, 2 * W + 1 : 2 * W + 1 + L])                 # Xi3

                # hann conv: xw_p = X_p - 0.5*(X_{p-1} + X_{p+1})
                # cross-plane neighbors (p0's f-1 = p3[q-1], p3's f+1 = p0[q+1])
                # via PE shift-matrix matmuls into PSUM.
                xw = p_xw.tile([128, NTILE * W], f16, tag="xw")
                M, A = ALU.mult, ALU.add

                def XT(t0):  # single xbf tile
                    return xbf[:, t0 * W : (t0 + 1) * W]

                def RX(t0):  # 2-tile flat range
                    return xbf[:, t0 * W : (t0 + 2) * W]

                def WT(t0):
                    return xw[:, t0 * W : (t0 + 1) * W]

                def RW(t0):
                    return xw[:, t0 * W : (t0 + 2) * W]

                for comp, gps_planes in (("r", (1, 2)), ("i", (1,))):
                    p0 = TSTART[(comp, 0)]
                    p2 = TSTART[(comp, 2)]
                    p1 = TSTART[(comp, 1)]
                    p3 = TSTART[(comp, 3)]
                    # S3 = X_p3 shifted down (q-1); S3[0]=0 matches ref f=0
                    t3a = xsps.tile([128, W], f32, tag="xs")
                    nc.tensor.matmul(t3a[:, :], shv[:, SDN, :], XT(p3),
                                     start=True, stop=True)
                    t3b = xsps.tile([128, W], f32, tag="xs")
                    nc.tensor.matmul(t3b[:, :], shv[:, SDN, :], XT(p3 + 1),
                                     start=True, stop=False)
                    nc.tensor.matmul(t3b[:, :], shv[:, CFIRST, :], XT(p3),
                                     start=False, stop=True)
                    # conv p0 per tile (frees t3a before t0b allocation)
                    nc.vector.tensor_add(WT(p0), t3a[:, :], XT(p1))
                    nc.vector.scalar_tensor_tensor(
                        WT(p0), WT(p0), -0.5, XT(p0), M, A)
                    t0a = xsps.tile([128, W], f32, tag="xs")
                    nc.tensor.matmul(t0a[:, :], shv[:, SUP, :], XT(p0),
                                     start=True, stop=False)
                    nc.tensor.matmul(t0a[:, :], shv[:, CLAST, :], XT(p0 + 1),
                                     start=False, stop=True)
                    nc.vector.tensor_add(WT(p0 + 1), t3b[:, :], XT(p1 + 1))
                    nc.vector.scalar_tensor_tensor(
                        WT(p0 + 1), WT(p0 + 1), -0.5, XT(p0 + 1), M, A)
                    # S0 = X_p0 shifted up (q+1); q=256 (f=1024) dropped -> 0
                    t0b = xsps.tile([128, W], f32, tag="xs")
                    nc.tensor.matmul(t0b[:, :], shv[:, SUP, :], XT(p0 + 1),
                                     start=True, stop=True)
                    # conv p3 per tile
                    nc.vector.tensor_add(WT(p3), XT(p2), t0a[:, :])
                    nc.vector.scalar_tensor_tensor(
                        WT(p3), WT(p3), -0.5, XT(p3), M, A)
                    nc.vector.tensor_add(WT(p3 + 1), XT(p2 + 1), t0b[:, :])
                    nc.vector.scalar_tensor_tensor(
                        WT(p3 + 1), WT(p3 + 1), -0.5, XT(p3 + 1), M, A)
                    # conv p1/p2 (no shifts): flat 2-tile ops
                    for p, lo, hi, ctr, out in (
                        (1, RX(p0), RX(p2), RX(p1), RW(p1)),
                        (2, RX(p1), RX(p3), RX(p2), RW(p2)),
                    ):
                        eng = nc.gpsimd if p in gps_planes else nc.vector
                        eng.tensor_add(out, lo, hi)
                        nc.vector.scalar_tensor_tensor(
                            out, out, -0.5, ctr, M, A)

                # power
                pw = p_pw.tile([128, NTILE * W], f16, tag="pw")
                hsplit = 8 * W
                nc.vector.tensor_mul(
                    pw[:, 0:hsplit], xw[:, 0:hsplit], xw[:, 0:hsplit]
                )
                nc.scalar.activation(
                    pw[:, hsplit:], xw[:, hsplit:], ACTF.Square
                )

                # mel projection
                mp = mps.tile([128, W], f32, tag="mp")
                for j in range(NTILE):
                    nc.tensor.matmul(
                        mp[:, 0:V],
                        fbb[:, j * 128 : (j + 1) * 128],
                        pw[:, j * W : j * W + V],
                        start=(j == 0),
                        stop=(j == NTILE - 1),
                    )
                nc.scalar.copy(melb[:, co : co + V], mp[:, 0:V])

            tlens = (128, 128, 57)

            def emit_tail(b):
                melT = p_tail.tile([128, 3 * 128], f16, tag="melT")
                for k in range(3):
                    tl = tlens[k]
                    tp = tps.tile([128, 128], f16, tag="tp")
                    nc.tensor.transpose(
                        tp[0:tl, :],
                        melb[:, b * SBLK + k * 128 : b * SBLK + k * 128 + tl],
                        idb[:, :],
                    )
                    nc.vector.tensor_copy(
                        melT[0:tl, k * 128 : (k + 1) * 128], tp[0:tl, :]
                    )
                msp = msps.tile([128, T], f32, tag="ms")
                for k in range(3):
                    tl = tlens[k]
                    nc.tensor.matmul(
                        msp[:, :],
                        melT[0:tl, k * 128 : (k + 1) * 128],
                        ltv[0:tl, k, :],
                        start=(k == 0),
                        stop=(k == 2),
                    )
                e1 = p_tail.tile([128, T], f32, tag="e1")
                e2 = p_tail.tile([128, T], f32, tag="e2")
                e3 = p_tail.tile([128, T], f16, tag="e3")
                e4 = p_tail.tile([128, T], f32, tag="e4")
                e5 = p_tail.tile([128, T], f32, tag="e5")
                ob = p_tail.tile([128, T], f32, tag="ob")
                # e1 = ln(ms*SCM/SCL + EPS)
                nc.scalar.activation(
                    e1[:, :], msp[:, :], ACTF.Ln,
                    bias=bias_t[:, 0:1], scale=SCM / SCL,
                )
                # e2 = exp(-alpha*e1 + ln(SCE)) = SCE*(eps+m)^-alpha
                nc.scalar.activation(
                    e2[:, :], e1[:, :], ACTF.Exp,
                    bias=bias_t[:, 1:2], scale=-ALPHA,
                )
                nc.vector.tensor_mul(
                    e3[:, :], e2[:, :], melb[:, b * SBLK : b * SBLK + T]
                )
                # e4 = ln(e3*SCM/SCE + DELTA)
                nc.scalar.activation(
                    e4[:, :], e3[:, :], ACTF.Ln,
                    bias=bias_t[:, 2:3], scale=SCM / SCE,
                )
                nc.scalar.activation(e5[:, :], e4[:, :], ACTF.Exp, scale=R)
                nc.vector.tensor_scalar_add(ob[:, :], e5[:, :], -(DELTA ** R))
                nc.sync.dma_start(y_d.ap()[b], ob[:, :])

            saved = {}
            saved[0] = emit_dft(0, emit_prep(0))
            saved[1] = emit_dft(1, emit_prep(1))
            for ci in range(nch):
                emit_post(ci, *saved.pop(ci))
                for b in TAIL_AFTER[ci]:
                    emit_tail(b)
                if ci + 2 < nch:
                    saved[ci + 2] = emit_dft(ci + 2, emit_prep(ci + 2))

    if split:
        _split_multiwaits(nc)
    return nc


# ---------------------------------------------------------------- host side

_CACHE = {}


def _get_consts():
    if "consts" not in _CACHE:
        E, fb2, LT = _build_consts()
        e_h = np.ascontiguousarray(
            E.astype(np.float16).reshape(4, 128, NTILE * 128)
        )
        # fb tile layout: fb_h[p, j*128+m] = fb2[j*128+p, m]
        fb_h = np.ascontiguousarray(
            fb2.astype(np.float16)
            .reshape(NTILE, 128, 128)
            .transpose(1, 0, 2)
            .reshape(128, NTILE * 128)
        )
        lt_h = np.ascontiguousarray(
            LT.astype(np.float16).reshape(3, 128, T)
        )
        id_h = np.eye(128, dtype=np.float16)
        sdn = np.eye(128, k=1)
        cfirst = np.zeros((128, 128)); cfirst[127, 0] = 1.0
        sup = np.eye(128, k=-1)
        clast = np.zeros((128, 128)); clast[0, 127] = 1.0
        sh_h = np.ascontiguousarray(
            np.concatenate([sdn, cfirst, sup, clast], axis=1).astype(np.float16)
        )
        _CACHE["consts"] = (e_h, fb_h, lt_h, id_h, sh_h)
    return _CACHE["consts"]


def _prep_core_input(wf_core):
    """wf_core: [BL, 160000] f32 -> xt2 [128, nch*4*SEG] f16.

    Hop-blocked transpose, staged per chunk: xt2[p, ci*4*SEG + rc*SEG + c]
    = x[rc*128 + p, co_ci + c] (chunk overlap duplicated, tails zero).
    """
    BL = wf_core.shape[0]
    x = np.pad(wf_core.astype(np.float32), ((0, 0), (PAD, PAD)), mode="reflect")
    blocks = x[:, : SBLK * HOP].reshape(BL, SBLK, HOP)
    xT = blocks.transpose(2, 0, 1).reshape(HOP, BL * SBLK).astype(np.float16)
    nch = len(CHUNKS)
    g = np.zeros((HOP, NC + SEG), np.float16)
    g[:, :NC] = xT
    z = np.empty((nch, 4, 128, SEG), np.float16)
    for ci, (co, w) in enumerate(CHUNKS):
        z[ci] = g[:, co : co + SEG].reshape(4, 128, SEG)
    xt2 = np.ascontiguousarray(
        z.transpose(2, 0, 1, 3).reshape(128, nch * 4 * SEG)
    )
    return xt2


def make_in_maps(waveform):
    e_h, fb_h, lt_h, id_h, sh_h = _get_consts()
    BL = B_TOTAL // N_CORES
    in_maps = []
    for c in range(N_CORES):
        xt2 = _prep_core_input(waveform[c * BL : (c + 1) * BL])
        in_maps.append(
            {"xt2": xt2, "e": e_h, "fb": fb_h, "lt": lt_h, "idn": id_h,
             "sh": sh_h}
        )
    return in_maps


def _get_nc():
    if "nc" not in _CACHE:
        _CACHE["nc"] = build_nc(BL=8)
    return _CACHE["nc"]


def kernel(waveform: np.ndarray) -> np.ndarray:
    from concourse.bass_utils import run_bass_kernel_spmd

    waveform = np.asarray(waveform, np.float32)
    assert waveform.shape == (B_TOTAL, L_WAVE)
    in_maps = make_in_maps(waveform)
    nc = _get_nc()
    res = run_bass_kernel_spmd(nc, in_maps, core_ids=list(range(N_CORES)))
    BL = B_TOTAL // N_CORES
    out = np.empty((B_TOTAL, 1, N_MELS, T), np.float32)
    for c in range(N_CORES):
        y = np.asarray(res.results[c]["y"])  # [BL, 128, T]
        out[c * BL : (c + 1) * BL, 0] = y
    return out
